# revision 34
# baseline (speedup 1.0000x reference)
"""Trainium2 kernel for the algo/task performance-scan problem.

Restructuring: the lax.scan's only cross-step dependency is through the 64
scalars sig[:, lx[l]] read each step.  That scalar chain (O(A*L + L^2) work)
is computed on the host in float64.  Given the per-step coefficients
c[a,l] = eff[a] + s[a,l]*boost[a], the full field is a banded matmul

    result[a, l, t] = sum_{j<=l} mem[a]^(l-j) * c[a,j] * row_j[t]

(mem ~ 0.5-0.72, so terms with l-j > ~64 are below fp32 noise), followed by
sig = tanh(result / (2*diff))  (identity: 2*sigmoid(x)-1 = tanh(x/2)).

Precision: error-compensated bf16 split (R = Rh+Rl, G = Gh+Gl;
Rh@Gh + Rl@Gh + Rh@Gl accumulated in fp32 PSUM) gives ~2e-5 field error at
full bf16 PE speed; the fp16 output rounding (~2.4e-4) dominates.

Per core (8 algos): 192 matmuls [K=128, M=128 t, N=512] (~44us PE), tanh
on ACT with per-partition 1/(2*diff) scale (~33us), fp16 output in
[g, t, a, l] layout so each partition stores one 4KB contiguous run (the
host permutes back).  A dummy activation during the DMA lead-in
pre-loads the tanh table.  Sharding: 8 algos per core.
"""

import sys

sys.path.insert(0, "/opt/trn_rl_repo")

import numpy as np

A, T, L = 64, 1024, 512
NCORES = 8
ACORE = A // NCORES          # 8 algos per core
LT = 64                      # l-tile size
NLT = L // LT                # 8 l-tiles
NTB = T // 128               # 8 task blocks
NG = 2                       # psum groups per tb (4 l-tiles each)

_CACHE = {}


def _build_program():
    import concourse.tile as tile
    from concourse import bacc, mybir

    nc = bacc.Bacc("TRN2", target_bir_lowering=False, debug=False,
                   enable_asserts=False, num_devices=NCORES)
    f32 = mybir.dt.float32
    f16 = mybir.dt.float16
    bf16 = mybir.dt.bfloat16

    # Inputs are pre-packed per consumption half (g=0 uses R chunks
    # A0,B0,A1 + G tiles 0-3; g=1 the rest) so each half loads with ONE
    # DMA — each dma_start costs a flat ~650ns of serialized issue time
    # on the Sync engine, so few big DMAs beat many small ones.
    rh0_in = nc.dram_tensor("rh0", [3, 128, T], bf16,
                            kind="ExternalInput").ap()
    rh1_in = nc.dram_tensor("rh1", [4, 128, T], bf16,
                            kind="ExternalInput").ap()
    rl0_in = nc.dram_tensor("rl0", [3, 128, T], bf16,
                            kind="ExternalInput").ap()
    rl1_in = nc.dram_tensor("rl1", [4, 128, T], bf16,
                            kind="ExternalInput").ap()
    gh0_in = nc.dram_tensor("gh0", [4, 128, ACORE * LT], bf16,
                            kind="ExternalInput").ap()
    gh1_in = nc.dram_tensor("gh1", [4, 128, ACORE * LT], bf16,
                            kind="ExternalInput").ap()
    gl0_in = nc.dram_tensor("gl0", [4, 128, ACORE * LT], bf16,
                            kind="ExternalInput").ap()
    gl1_in = nc.dram_tensor("gl1", [4, 128, ACORE * LT], bf16,
                            kind="ExternalInput").ap()
    d_in = nc.dram_tensor("d", [128, NTB], f32, kind="ExternalInput").ap()
    # [g, t, a, l-within-group] so each partition's store is one 4KB
    # contiguous run; the host permutes back to [a, t, l].
    out = nc.dram_tensor("out", [NG, T, ACORE, 256], f16,
                         kind="ExternalOutput").ap()

    # R chunk per l-tile: window j in [js, js+127], js = 0 if lt==0 else
    # 64*(lt-1).  Even-aligned windows (odd lt, and lt=0) come from "A"
    # chunks at j = 0,128,256,384; odd-aligned (even lt>=2) from "B"
    # chunks at j = 64,192,320.
    chunk_specs = [("A0", 0), ("A1", 128), ("A2", 256), ("A3", 384),
                   ("B0", 64), ("B1", 192), ("B2", 320)]
    lt_chunk = ["A0", "A0", "B0", "A1", "B1", "A2", "B2", "A3"]
    chunk_js = dict(chunk_specs)

    with tile.TileContext(nc) as tc:
        with tc.tile_pool(name="consts", bufs=1) as consts, \
             tc.tile_pool(name="outp", bufs=6) as outp, \
             tc.tile_pool(name="ps", bufs=2, space="PSUM") as psp:

            # Pre-load the tanh ACT table during the input-DMA lead-in so
            # the first real activation doesn't pay the ~1.3us table load.
            wsrc = consts.tile([128, 64], bf16, tag="warm")
            wdst = consts.tile([128, 64], f16, tag="warmout")
            nc.gpsimd.memset(wsrc[:], 0.0)
            nc.scalar.activation(wdst[:], wsrc[:],
                                 mybir.ActivationFunctionType.Tanh,
                                 scale=1.0)

            def bulk(tag, src, n, width):
                t_ = consts.tile([128, n * width], bf16, tag=tag)
                nc.sync.dma_start(
                    t_[:].rearrange("p (c w) -> p c w", c=n), src)
                return t_

            # g=0 operand set first, dsc between the halves (first needed
            # by the first ACT, ~14us in)
            rh0 = bulk("rh0", rh0_in.rearrange("c p w -> p c w"), 3, T)
            gh0 = bulk("gh0", gh0_in.rearrange("c p w -> p c w"), 4,
                       ACORE * LT)
            gl0 = bulk("gl0", gl0_in.rearrange("c p w -> p c w"), 4,
                       ACORE * LT)
            rl0 = bulk("rl0", rl0_in.rearrange("c p w -> p c w"), 3, T)
            dsc = consts.tile([128, NTB], f32, tag="dsc")
            nc.sync.dma_start(dsc[:], d_in[:])
            rh1 = bulk("rh1", rh1_in.rearrange("c p w -> p c w"), 4, T)
            gh1 = bulk("gh1", gh1_in.rearrange("c p w -> p c w"), 4,
                       ACORE * LT)
            gl1 = bulk("gl1", gl1_in.rearrange("c p w -> p c w"), 4,
                       ACORE * LT)
            rl1 = bulk("rl1", rl1_in.rearrange("c p w -> p c w"), 4, T)

            chunk_pos = {"A0": (0, 0), "B0": (0, 1), "A1": (0, 2),
                         "B1": (1, 0), "A2": (1, 1), "B2": (1, 2),
                         "A3": (1, 3)}
            rt = {}
            for name, (half, idx) in chunk_pos.items():
                rh_t = (rh0, rh1)[half]
                rl_t = (rl0, rl1)[half]
                rt[name] = (rh_t[:, idx * T:(idx + 1) * T],
                            rl_t[:, idx * T:(idx + 1) * T])
            W = ACORE * LT
            gt = {lt: ((gh0, gh1)[lt // 4][:, (lt % 4) * W:(lt % 4 + 1) * W],
                       (gl0, gl1)[lt // 4][:, (lt % 4) * W:(lt % 4 + 1) * W])
                  for lt in range(NLT)}

            for g in range(NG):
                for tb in range(NTB):
                    ps = psp.tile([128, 4 * 512], f32, tag="ps")
                    for sub in range(4):
                        lt = g * 4 + sub
                        rh_t, rl_t = rt[lt_chunk[lt]]
                        gh_t, gl_t = gt[lt]
                        pslice = ps[:, sub * 512:(sub + 1) * 512]
                        lhs_h = rh_t[:, tb * 128:(tb + 1) * 128]
                        lhs_l = rl_t[:, tb * 128:(tb + 1) * 128]
                        nc.tensor.matmul(pslice, lhsT=lhs_h, rhs=gh_t[:],
                                         start=True, stop=False)
                        nc.tensor.matmul(pslice, lhsT=lhs_h, rhs=gl_t[:],
                                         start=False, stop=False)
                        nc.tensor.matmul(pslice, lhsT=lhs_l, rhs=gh_t[:],
                                         start=False, stop=True)
                    # psum free layout: s*512 + a*64 + ll
                    # osb free layout:  a*256 + s*64 + ll
                    osb = outp.tile([128, ACORE * 256], f16, tag="osb")
                    nc.scalar.activation(
                        osb[:].rearrange("p (a s l) -> p s a l", a=ACORE,
                                         s=4),
                        ps[:].rearrange("p (s a l) -> p s a l", s=4,
                                        a=ACORE),
                        mybir.ActivationFunctionType.Tanh,
                        scale=dsc[:, tb:tb + 1])
                    nc.sync.dma_start(
                        out[g, tb * 128:(tb + 1) * 128],
                        osb[:].rearrange("p (a l) -> p a l", a=ACORE))

    nc.compile()
    return nc


def _host_chain(lx, task_matrix, task_difficulty, alg_efficiency,
                alg_memory, alg_experience_boost):
    """Exact (f64) scalar feedback chain + banded coefficient tensors."""
    import ml_dtypes
    bf = ml_dtypes.bfloat16

    lx = np.asarray(lx).astype(np.int64)
    TM = np.asarray(task_matrix, dtype=np.float64)
    diff = np.asarray(task_difficulty, dtype=np.float64)
    eff = np.asarray(alg_efficiency, dtype=np.float64)
    mem = np.asarray(alg_memory, dtype=np.float64)
    boost = np.asarray(alg_experience_boost, dtype=np.float64)

    R = TM[lx]                     # [L, T]
    TM2 = R[:, lx]                 # [L, L]
    dlx = diff[lx]                 # [L]

    resS = np.zeros((A, L))
    c = np.empty((A, L))
    for l in range(L):
        s_l = 2.0 / (1.0 + np.exp(-resS[:, l] / dlx[l])) - 1.0
        c[:, l] = eff + s_l * boost
        resS = resS * mem[:, None] + c[:, l][:, None] * TM2[l][None, :]

    Rf = R.astype(np.float32)
    Rh = Rf.astype(bf)
    Rl = (Rf - Rh.astype(np.float32)).astype(bf)

    # G[a, lt, jj, ll] = mem^(l-j) * c[a, j], j = js(lt)+jj, l = 64*lt+ll
    pmat = mem[:, None] ** np.arange(192)[None, :]       # [A, 192]
    G = np.zeros((A, NLT, 128, LT), dtype=np.float64)
    for lt in range(NLT):
        js = 0 if lt == 0 else 64 * (lt - 1)
        jw = np.arange(js, js + 128)
        lmj = (np.arange(LT)[None, :] + 64 * lt) - jw[:, None]   # [128, LT]
        valid = lmj >= 0
        G[:, lt] = np.where(valid[None],
                            pmat[:, np.maximum(lmj, 0)] * c[:, jw][:, :, None],
                            0.0)
    Gf = G.astype(np.float32)
    Gh = Gf.astype(bf)
    Gl = (Gf - Gh.astype(np.float32)).astype(bf)

    def pack(Gx):
        packs = []
        for core in range(NCORES):
            blk = Gx[core * ACORE:(core + 1) * ACORE]    # [ACORE,NLT,128,LT]
            packs.append(np.ascontiguousarray(
                blk.transpose(1, 2, 0, 3).reshape(NLT, 128, ACORE * LT)))
        return packs

    def rpack(Rx, starts):
        return np.ascontiguousarray(
            np.stack([Rx[s:s + 128] for s in starts]))

    r0s, r1s = [0, 64, 128], [192, 256, 320, 384]
    rpk = {"rh0": rpack(Rh, r0s), "rh1": rpack(Rh, r1s),
           "rl0": rpack(Rl, r0s), "rl1": rpack(Rl, r1s)}
    gh_packs, gl_packs = pack(Gh), pack(Gl)
    gpk = [{"gh0": np.ascontiguousarray(gh_packs[c][:4]),
            "gh1": np.ascontiguousarray(gh_packs[c][4:]),
            "gl0": np.ascontiguousarray(gl_packs[c][:4]),
            "gl1": np.ascontiguousarray(gl_packs[c][4:])}
           for c in range(NCORES)]

    dsc = np.ascontiguousarray(
        (1.0 / (2.0 * diff)).reshape(NTB, 128).T).astype(np.float32)
    return rpk, gpk, dsc


def kernel(lx, task_matrix, task_difficulty, alg_efficiency, alg_memory,
           alg_experience_boost):
    from concourse.bass_utils import run_bass_kernel_spmd

    rpk, gpk, dsc = _host_chain(
        lx, task_matrix, task_difficulty, alg_efficiency, alg_memory,
        alg_experience_boost)

    if "nc" not in _CACHE:
        _CACHE["nc"] = _build_program()
    nc = _CACHE["nc"]

    in_maps = [{**rpk, **gpk[c], "d": dsc} for c in range(NCORES)]
    res = run_bass_kernel_spmd(nc, in_maps, core_ids=list(range(NCORES)),
                               trace=False)
    out = np.empty((A, T, L + 1), dtype=np.float32)
    out[:, :, 0] = 0.0
    for c in range(NCORES):
        dev = res.results[c]["out"]          # [NG, T, ACORE, 256] f16
        out[c * ACORE:(c + 1) * ACORE, :, 1:] = (
            dev.transpose(2, 1, 0, 3).reshape(ACORE, T, L).astype(np.float32))
    return out
